# revision 5
# baseline (speedup 1.0000x reference)
"""ContrastiveTokenLoss on Trainium2 (8 NeuronCores, Bass/Tile) — v2.2.

Problem (hardcoded): input [2, 2048, 32000] f32 logits, target [2, 2048] int.
ct_len = 512, win = 256, IGNORE_INDEX = -100, PAD_ID = 0.

loss = sum_{b, i<512} valid(b,i) * log1p( sum_{j in [i-256, i), tgt[b,j]!=0}
           exp(x[b,i,tgt[b,j]] - x[b,i,tgt_safe[b,i]]) ) / max(#valid, 1)

Sharding (v2): one (batch, 128-position block) per core — core k handles
batch k//4, positions [128*(k%4), 128*(k%4)+128).  The block's window
tokens span 384 slots ([s-256, s+128)); slot t = c*128+p is row p of gather
chunk c (3 chunks vs the old layout's 5: a same-batch position block shares
2.5x fewer window tokens, and chunk 1 is fully in-band so only chunks 0/2
need masks).  Logits are staged vocab-major ([32001, 128] f32; row 32000 is
a -1e9 sentinel for out-of-range / PAD window tokens).

On device: 3 indirect row-gathers (diag chunk first) -> PE transposes into
ONE PSUM bank ([q, slot] = x[q, tok_slot] + additive band mask, the mask
preloaded on the transpose datapath and accumulated over) -> diag-extract
of -pos (DVE mult-by-identity + negated row-reduce) -> ONE 384-wide exp
(bias = -pos) with row accumulation -> DMA out r [128, 1] f32, where
r = 1 (unmasked diagonal's exp(0)) + sum of banded exps.  The host computes
sum(valid*ln(r))/max(n,1) in f64 — no Ln / partition-reduce on device.  An
early dummy exp pulls the ACT table load off the critical path.
"""

import numpy as np
from contextlib import ExitStack

import concourse.bass as bass
import concourse.bacc as bacc
import concourse.mybir as mybir
import concourse.tile as tile
from concourse.bass_utils import run_bass_kernel_spmd

B, T, V = 2, 2048, 32000
CT = 512
WIN = 256
IGNORE_INDEX = -100
PAD_ID = 0
NCORES = 8
CI = 128                   # positions per core (one batch each)
SENT = V                   # sentinel row index (-1e9)
F32 = mybir.dt.float32
I32 = mybir.dt.int32

_CACHE = {}


def _build(pos_chunk):
    """pos_chunk=False (fast, no PAD/ignore targets in range): 3 chunks, pos
    read off the window diagonal of chunk 2 (kept unmasked: its exp(0)=1 is
    log1p's "+1").  pos_chunk=True: a 4th chunk holds the clamped own-target
    rows (all masked except the diagonal; PAD window copies are sentineled,
    chunk 2's diagonal is then banned by the band mask)."""
    nch = 4 if pos_chunk else 3
    W = nch * CI
    dg = 2 + int(pos_chunk)          # diag-carrying chunk
    masked = [0, 2, 3] if pos_chunk else [0, 2]   # chunk 1 is fully in-band
    nc = bacc.Bacc("TRN2", target_bir_lowering=False)
    xt = nc.dram_tensor("xt", [V + 1, CI], F32, kind="ExternalInput")
    idx = nc.dram_tensor("idx", [CI, nch], I32, kind="ExternalInput")
    cst = nc.dram_tensor("cst", [CI, (len(masked) + 1) * CI], F32,
                         kind="ExternalInput")
    out = nc.dram_tensor("out", [CI, 1], F32, kind="ExternalOutput")

    with ExitStack() as ctx:
        tc = ctx.enter_context(tile.TileContext(nc))
        sb = ctx.enter_context(tc.tile_pool(name="sb", bufs=1))
        ps = ctx.enter_context(tc.tile_pool(name="ps", bufs=1, space="PSUM"))

        # gather row indices first so the gathers start as early as possible
        it = sb.tile([CI, nch], I32)
        nc.sync.dma_start(it[:], idx[:])

        # dummy exp on a zeroed tile: hoists the ACT table load to t~0 (it
        # otherwise lands on the critical path right before the real exp)
        z = sb.tile([CI, 2], F32)
        nc.vector.memset(z[:, 0:1], 0.0)
        nc.scalar.activation(
            z[:, 1:2], z[:, 0:1], mybir.ActivationFunctionType.Exp,
            bias=z[:, 0:1], scale=1.0,
        )

        cst_sb = sb.tile([CI, (len(masked) + 1) * CI], F32)
        nc.sync.dma_start(cst_sb[:], cst[:])
        mb = {c: cst_sb[:, i * CI : (i + 1) * CI] for i, c in enumerate(masked)}
        ident = cst_sb[:, len(masked) * CI :]

        # indirect row gathers, diag-carrying chunk first: descriptor p of
        # chunk c reads the 512B row xt[idx[p, c], :] into gts[c][p, :]
        order = [dg] + [c for c in range(nch) if c != dg]
        gts = {}
        for c in order:
            gt = sb.tile([CI, CI], F32, tag=f"gt{c}")
            gts[c] = gt
            nc.gpsimd.indirect_dma_start(
                out=gt[:],
                out_offset=None,
                in_=xt[:],
                in_offset=bass.IndirectOffsetOnAxis(ap=it[:, c : c + 1], axis=0),
            )

        # PE: per chunk, preload the additive band mask into its PSUM column
        # range on the transpose datapath (blocks stored pre-transposed),
        # then ACCUMULATE the gather's transpose on top:
        #   pt[q, 128c+p] = x[q, window slot 128c+p] + mask[q, 128c+p]
        pt = ps.tile([CI, W], F32, space="PSUM")
        for c in order:
            sl = slice(c * CI, (c + 1) * CI)
            if c in mb:
                nc.tensor.matmul(
                    out=pt[:, sl], lhsT=mb[c], rhs=ident,
                    is_transpose=True, start=True, stop=False,
                )
                nc.tensor.matmul(
                    out=pt[:, sl], lhsT=gts[c][:], rhs=ident,
                    is_transpose=True, start=False, stop=True,
                )
            else:
                nc.tensor.transpose(out=pt[:, sl], in_=gts[c][:], identity=ident)

        # npos[q] = -pt[q, dg*128+q] = -pos (diag is unmasked there)
        pd = sb.tile([CI, CI], F32)
        npos = sb.tile([CI, 1], F32)
        nc.vector.tensor_tensor(
            pd[:], pt[:, dg * CI : (dg + 1) * CI], ident, mybir.AluOpType.mult
        )
        nc.vector.reduce_sum(
            npos[:], pd[:], axis=mybir.AxisListType.X, negate=True
        )

        # r = sum_slot exp(pt[q, slot] - pos[q]); masked slots give exp(-1e9)
        # = 0, the unmasked diagonal gives exp(0) = 1 == log1p's "+1"
        e = sb.tile([CI, W], F32)
        r = sb.tile([CI, 1], F32)
        nc.scalar.activation(
            e[:], pt[:], mybir.ActivationFunctionType.Exp,
            bias=npos[:], scale=1.0, accum_out=r[:],
        )
        nc.sync.dma_start(out[:], r[:])
    nc.compile()
    return nc


def _get_nc(pos_chunk):
    key = f"nc{pos_chunk}"
    if key not in _CACHE:
        _CACHE[key] = _build(pos_chunk)
    return _CACHE[key]


def _consts(pos_chunk):
    key = f"cst{pos_chunk}"
    if key not in _CACHE:
        masked = [0, 2, 3] if pos_chunk else [0, 2]
        dg = 2 + int(pos_chunk)
        q = np.arange(CI)[:, None]
        cstv = np.empty((CI, (len(masked) + 1) * CI), np.float32)
        for i, c in enumerate(masked):
            slot = c * CI + np.arange(CI)[None, :]
            m = np.where((slot >= q) & (slot < q + WIN), 0.0, -1e9)
            if c == dg:
                # own-token diagonal stays unmasked: exp(0) = 1 is the "+1"
                m[np.arange(CI), np.arange(CI)] = 0.0
            # stored transposed: the preload runs on the transpose datapath
            cstv[:, i * CI : (i + 1) * CI] = m.T
        cstv[:, len(masked) * CI :] = np.eye(CI, dtype=np.float32)
        _CACHE[key] = np.ascontiguousarray(cstv)
    return _CACHE[key]


def kernel(input, target, _trace=False):
    input = np.asarray(input, dtype=np.float32)
    target = np.asarray(target)
    t32 = target[:, :CT].astype(np.int32)

    # fast path iff no in-range target is PAD (0), negative, or ignore
    pos_chunk = bool((t32 <= 0).any())
    nch = 4 if pos_chunk else 3
    cstv = _consts(pos_chunk)

    in_maps = []
    for k in range(NCORES):
        b, s = k // 4, CI * (k % 4)
        xtk = np.empty((V + 1, CI), np.float32)
        xtk[:V] = input[b, s : s + CI].T
        xtk[V:] = -1e9

        j = (s - WIN) + np.arange(3 * CI)          # window token positions
        tj = t32[b, np.clip(j, 0, CT - 1)]
        win_ids = np.where(j < 0, SENT, tj)
        if pos_chunk:
            win_ids = np.where((j >= 0) & (tj <= PAD_ID), SENT, win_ids)
            own = np.maximum(t32[b, s : s + CI], 0)
            ids_flat = np.concatenate([win_ids, own])
        else:
            ids_flat = win_ids
        # idx[p, c] = row for slot c*128+p
        idxs = np.ascontiguousarray(
            ids_flat.reshape(nch, CI).T.astype(np.int32)
        )
        in_maps.append({"xt": xtk, "idx": idxs, "cst": cstv})

    nc = _get_nc(pos_chunk)
    br = run_bass_kernel_spmd(
        nc, in_maps, core_ids=list(range(NCORES)), trace=_trace
    )
    rs = np.stack([r["out"][:, 0] for r in br.results])   # [8, 128]
    valid = (t32 != IGNORE_INDEX)                          # [2, 512]
    vs = valid.reshape(2, 4, CI)[[0, 0, 0, 0, 1, 1, 1, 1],
                                 [0, 1, 2, 3, 0, 1, 2, 3]] # [8, 128]
    loss = (np.log(rs.astype(np.float64)) * vs).sum()
    denom = max(valid.sum(), 1)
    kernel.last_results = br
    return np.asarray(np.float32(loss / denom))


# revision 8
# speedup vs baseline: 1.2339x; 1.2339x over previous
"""ContrastiveTokenLoss on Trainium2 (8 NeuronCores, Bass/Tile) — v2.2.

Problem (hardcoded): input [2, 2048, 32000] f32 logits, target [2, 2048] int.
ct_len = 512, win = 256, IGNORE_INDEX = -100, PAD_ID = 0.

loss = sum_{b, i<512} valid(b,i) * log1p( sum_{j in [i-256, i), tgt[b,j]!=0}
           exp(x[b,i,tgt[b,j]] - x[b,i,tgt_safe[b,i]]) ) / max(#valid, 1)

Sharding (v2): one (batch, 128-position block) per core — core k handles
batch k//4, positions [128*(k%4), 128*(k%4)+128).  The block's window
tokens span 384 slots ([s-256, s+128)); slot t = c*128+p is row p of gather
chunk c (3 chunks vs the old layout's 5: a same-batch position block shares
2.5x fewer window tokens, and chunk 1 is fully in-band so only chunks 0/2
need masks).  Logits are staged vocab-major ([32001, 128] f32; row 32000 is
a -1e9 sentinel for out-of-range / PAD window tokens).

On device: 3 indirect row-gathers (diag chunk first) -> PE transposes into
ONE PSUM bank ([q, slot] = x[q, tok_slot] + additive band mask, the mask
preloaded on the transpose datapath and accumulated over) -> diag-extract
of -pos (DVE mult-by-identity + negated row-reduce) -> ONE 384-wide exp
(bias = -pos) with row accumulation -> DMA out r [128, 1] f32, where
r = 1 (unmasked diagonal's exp(0)) + sum of banded exps.  The host computes
sum(valid*ln(r))/max(n,1) in f64 — no Ln / partition-reduce on device.  An
early dummy exp pulls the ACT table load off the critical path.
"""

import numpy as np
from contextlib import ExitStack

import concourse.bass as bass
import concourse.bacc as bacc
import concourse.mybir as mybir
import concourse.tile as tile
from concourse.bass_utils import run_bass_kernel_spmd

B, T, V = 2, 2048, 32000
CT = 512
WIN = 256
IGNORE_INDEX = -100
PAD_ID = 0
NCORES = 8
CI = 128                   # positions per core (one batch each)
SENT = V                   # sentinel row index (-1e9)
F32 = mybir.dt.float32
I32 = mybir.dt.int32

_CACHE = {}


def _build(pos_chunk):
    """pos_chunk=False (fast, no PAD/ignore targets in range): 3 chunks, pos
    read off the window diagonal of chunk 2 (kept unmasked: its exp(0)=1 is
    log1p's "+1").  pos_chunk=True: a 4th chunk holds the clamped own-target
    rows (all masked except the diagonal; PAD window copies are sentineled,
    chunk 2's diagonal is then banned by the band mask)."""
    nch = 4 if pos_chunk else 3
    W = nch * CI
    dg = 2 + int(pos_chunk)          # diag-carrying chunk
    masked = [0, 2, 3] if pos_chunk else [0, 2]   # chunk 1 is fully in-band
    nc = bacc.Bacc("TRN2", target_bir_lowering=False)
    xt = nc.dram_tensor("xt", [V + 1, CI], F32, kind="ExternalInput")
    idx = nc.dram_tensor("idx", [CI, nch], I32, kind="ExternalInput")
    cst = nc.dram_tensor("cst", [CI, (len(masked) + 1) * CI], F32,
                         kind="ExternalInput")
    out = nc.dram_tensor("out", [1, CI], F32, kind="ExternalOutput")

    with ExitStack() as ctx:
        tc = ctx.enter_context(tile.TileContext(nc))
        sb = ctx.enter_context(tc.tile_pool(name="sb", bufs=1))
        ps = ctx.enter_context(tc.tile_pool(name="ps", bufs=1, space="PSUM"))

        # gather row indices first so the gathers start as early as possible
        it = sb.tile([CI, nch], I32)
        nc.sync.dma_start(it[:], idx[:])

        # dummy exp on a zeroed tile: hoists the ACT table load to t~0 (it
        # otherwise lands on the critical path right before the real exp)
        z = sb.tile([CI, 2], F32)
        nc.vector.memset(z[:, 0:1], 0.0)
        nc.scalar.activation(
            z[:, 1:2], z[:, 0:1], mybir.ActivationFunctionType.Exp,
            bias=z[:, 0:1], scale=1.0,
        )

        cst_sb = sb.tile([CI, (len(masked) + 1) * CI], F32)
        nc.sync.dma_start(cst_sb[:], cst[:])
        mb = {c: cst_sb[:, i * CI : (i + 1) * CI] for i, c in enumerate(masked)}
        ident = cst_sb[:, len(masked) * CI :]

        # indirect row gathers, diag-carrying chunk first: descriptor p of
        # chunk c reads the 512B row xt[idx[p, c], :] into gts[c][p, :]
        order = [dg] + [c for c in range(nch) if c != dg]
        gts = {}
        for c in order:
            gt = sb.tile([CI, CI], F32, tag=f"gt{c}")
            gts[c] = gt
            nc.gpsimd.indirect_dma_start(
                out=gt[:],
                out_offset=None,
                in_=xt[:],
                in_offset=bass.IndirectOffsetOnAxis(ap=it[:, c : c + 1], axis=0),
            )

        # PE: per chunk, preload the additive band mask into its PSUM column
        # range on the transpose datapath (blocks stored pre-transposed),
        # then ACCUMULATE the gather's transpose on top:
        #   pt[q, 128c+p] = x[q, window slot 128c+p] + mask[q, 128c+p]
        pt = ps.tile([CI, W], F32, space="PSUM")
        for c in order:
            sl = slice(c * CI, (c + 1) * CI)
            if c in mb:
                nc.tensor.matmul(
                    out=pt[:, sl], lhsT=mb[c], rhs=ident,
                    is_transpose=True, start=True, stop=False,
                )
                nc.tensor.matmul(
                    out=pt[:, sl], lhsT=gts[c][:], rhs=ident,
                    is_transpose=True, start=False, stop=True,
                )
            else:
                nc.tensor.transpose(out=pt[:, sl], in_=gts[c][:], identity=ident)

        # npos[q] = -pt[q, dg*128+q] = -pos (diag is unmasked there)
        pd = sb.tile([CI, CI], F32)
        npos = sb.tile([CI, 1], F32)
        nc.vector.tensor_tensor(
            pd[:], pt[:, dg * CI : (dg + 1) * CI], ident, mybir.AluOpType.mult
        )
        nc.vector.reduce_sum(
            npos[:], pd[:], axis=mybir.AxisListType.X, negate=True
        )

        # r = sum_slot exp(pt[q, slot] - pos[q]); masked slots give exp(-1e9)
        # = 0, the unmasked diagonal gives exp(0) = 1 == log1p's "+1"
        e = sb.tile([CI, W], F32)
        r = sb.tile([CI, 1], F32)
        nc.scalar.activation(
            e[:], pt[:], mybir.ActivationFunctionType.Exp,
            bias=npos[:], scale=1.0, accum_out=r[:],
        )

        # r is one 4B scalar per partition; DMAing that directly is 128
        # 4-byte HBM writes whose completion receipts cost ~6us.  Transpose
        # to one partition (PE), copy PSUM->SBUF (DVE), DMA one 512B row.
        rt = ps.tile([1, CI], F32, tag="rt", space="PSUM")
        nc.tensor.transpose(out=rt[:], in_=r[:], identity=ident)
        ro = sb.tile([1, CI], F32)
        nc.vector.tensor_scalar_add(ro[:], rt[:], 0.0)
        nc.sync.dma_start(out[:], ro[:])
    nc.compile()
    return nc


def _get_nc(pos_chunk):
    key = f"nc{pos_chunk}"
    if key not in _CACHE:
        _CACHE[key] = _build(pos_chunk)
    return _CACHE[key]


def _consts(pos_chunk):
    key = f"cst{pos_chunk}"
    if key not in _CACHE:
        masked = [0, 2, 3] if pos_chunk else [0, 2]
        dg = 2 + int(pos_chunk)
        q = np.arange(CI)[:, None]
        cstv = np.empty((CI, (len(masked) + 1) * CI), np.float32)
        for i, c in enumerate(masked):
            slot = c * CI + np.arange(CI)[None, :]
            m = np.where((slot >= q) & (slot < q + WIN), 0.0, -1e9)
            if c == dg:
                # own-token diagonal stays unmasked: exp(0) = 1 is the "+1"
                m[np.arange(CI), np.arange(CI)] = 0.0
            # stored transposed: the preload runs on the transpose datapath
            cstv[:, i * CI : (i + 1) * CI] = m.T
        cstv[:, len(masked) * CI :] = np.eye(CI, dtype=np.float32)
        _CACHE[key] = np.ascontiguousarray(cstv)
    return _CACHE[key]


def kernel(input, target, _trace=False):
    input = np.asarray(input, dtype=np.float32)
    target = np.asarray(target)
    t32 = target[:, :CT].astype(np.int32)

    # fast path iff no in-range target is PAD (0), negative, or ignore
    pos_chunk = bool((t32 <= 0).any())
    nch = 4 if pos_chunk else 3
    cstv = _consts(pos_chunk)

    in_maps = []
    for k in range(NCORES):
        b, s = k // 4, CI * (k % 4)
        xtk = np.empty((V + 1, CI), np.float32)
        xtk[:V] = input[b, s : s + CI].T
        xtk[V:] = -1e9

        j = (s - WIN) + np.arange(3 * CI)          # window token positions
        tj = t32[b, np.clip(j, 0, CT - 1)]
        win_ids = np.where(j < 0, SENT, tj)
        if pos_chunk:
            win_ids = np.where((j >= 0) & (tj <= PAD_ID), SENT, win_ids)
            own = np.maximum(t32[b, s : s + CI], 0)
            ids_flat = np.concatenate([win_ids, own])
        else:
            ids_flat = win_ids
        # idx[p, c] = row for slot c*128+p
        idxs = np.ascontiguousarray(
            ids_flat.reshape(nch, CI).T.astype(np.int32)
        )
        in_maps.append({"xt": xtk, "idx": idxs, "cst": cstv})

    nc = _get_nc(pos_chunk)
    br = run_bass_kernel_spmd(
        nc, in_maps, core_ids=list(range(NCORES)), trace=_trace
    )
    rs = np.stack([r["out"][0] for r in br.results])      # [8, 128]
    valid = (t32 != IGNORE_INDEX)                          # [2, 512]
    vs = valid.reshape(2, 4, CI)[[0, 0, 0, 0, 1, 1, 1, 1],
                                 [0, 1, 2, 3, 0, 1, 2, 3]] # [8, 128]
    loss = (np.log(rs.astype(np.float64)) * vs).sum()
    denom = max(valid.sum(), 1)
    kernel.last_results = br
    return np.asarray(np.float32(loss / denom))
